# revision 1
# baseline (speedup 1.0000x reference)
"""DeepFM forward on 8 Trainium2 NeuronCores (Bass/Tile).

Strategy: data-parallel batch split (2048 rows/core), embedding + first-order
tables combined into one [F*V, 17] f32 table replicated on every core (no
collectives).  Per 256-row pair-tile: 52 per-field SWDGE indirect row-gathers
(the HW indirect DMA pairs one index per destination partition; multi-index
offset APs are not honored by the TRN2 ucode), FM first/second order on
DVE+ACT with pair-widened 3D access patterns, PE transposes to feature-major;
then an fp32r (FP22, 1 cycle/row) PE MLP with biases, continuous linear term
and the FM output folded in via host-prepared (FP22-rounded) weights.  MLP for
batch-tile g is emitted right after its row-tiles so PE matmuls overlap the
next group's gathers; the Pool engine (SWDGE descriptor generation, ~1us per
128-descriptor call) is the critical path at ~440us/core.
"""
import sys
sys.path.insert(0, "/opt/trn_rl_repo")
import numpy as np
import jax
from jax.sharding import Mesh, PartitionSpec, NamedSharding
from jax.experimental.shard_map import shard_map

from concourse import bass, bacc, tile, mybir
from concourse.bass2jax import install_neuronx_cc_hook, _bass_exec_p, partition_id_tensor
from concourse.masks import make_identity

F32 = mybir.dt.float32
F32R = mybir.dt.float32r
I32 = mybir.dt.int32
AF = mybir.ActivationFunctionType
ALU = mybir.AluOpType

F, V, E = 26, 100000, 16
CONT, H1, H2 = 13, 400, 400
B = 16384
NCORES = 8
BC = B // NCORES          # 2048 rows per core
NT = BC // 128            # 16 tiles of 128 rows
NP = NT // 2              # 8 pair-tiles of 256 rows
NBT = BC // 512           # 4 matmul b-tiles of 512
GW = 17                   # combined row width (16 emb + 1 first)
HD = 17                   # header cols: 13 cont, ones, fm, 2 pad
GF = HD + F * GW          # 459 valid G columns
GP = 512                  # padded per-tile G width
REPEAT = 1                # program-level repeats (bench only; kernel() uses 1)
K1 = [(0, 128), (128, 256), (256, 384), (384, GF)]        # L1 K chunks
M1 = [(0, 128), (128, 256), (256, 384), (384, 401)]       # H chunks (L1 adds ones col)
M2 = [(0, 128), (128, 256), (256, 384), (384, 400)]       # L2 H chunks
K2 = [(0, 128), (128, 256), (256, 384), (384, 401)]       # L2 K chunks (400+ones)


def _view(ap, col, pair_stride, npair, width):
    """3D AP view: [128 partitions, npair pairs (stride pair_stride), width]."""
    pstep = ap.ap[0][0]
    return bass.AP(ap.tensor, ap.offset + col,
                   [(pstep, 128), (pair_stride, npair), (1, width)])


def _build():
    nc = bacc.Bacc("TRN2", target_bir_lowering=False, debug=False,
                   num_devices=NCORES)
    gtab = nc.dram_tensor("gtab", [F * V, GW], F32, kind="ExternalInput").ap()
    idx_d = nc.dram_tensor("idx_d", [BC, F], I32, kind="ExternalInput").ap()
    cont_d = nc.dram_tensor("cont_d", [BC, CONT], F32, kind="ExternalInput").ap()
    w1_d = nc.dram_tensor("w1_d", [GF, H1 + 1], F32R, kind="ExternalInput").ap()
    w2_d = nc.dram_tensor("w2_d", [401, H2], F32R, kind="ExternalInput").ap()
    wot_d = nc.dram_tensor("wot_d", [128, 4], F32R, kind="ExternalInput").ap()
    v0_d = nc.dram_tensor("v0_d", [128, 1], F32R, kind="ExternalInput").ap()
    out_d = nc.dram_tensor("out_d", [1, BC], F32, kind="ExternalOutput").ap()

    with tile.TileContext(nc) as tc:
        with (
            tc.tile_pool(name="per", bufs=1) as per,
            tc.tile_pool(name="gp", bufs=3) as gp,
            tc.tile_pool(name="fm", bufs=2) as fmp,
            tc.tile_pool(name="h1p", bufs=2) as h1p,
            tc.tile_pool(name="h2p", bufs=2) as h2p,
            tc.tile_pool(name="pst", bufs=2, space="PSUM") as pst,
            tc.tile_pool(name="psm", bufs=3, space="PSUM") as psm,
            tc.tile_pool(name="psf", bufs=2, space="PSUM") as psf,
        ):
            # ---- persistent weights / staging ----
            ident = per.tile([128, 128], F32, tag="ident")
            make_identity(nc, ident[:])
            w1c = []
            for ci, (a, b) in enumerate(K1):
                t = per.tile([b - a, H1 + 1], F32R, tag=f"w1c{ci}", name=f"w1c{ci}")
                nc.sync.dma_start(t[:], w1_d[a:b, :])
                w1c.append(t)
            w2c = []
            for ci, (a, b) in enumerate(K2):
                t = per.tile([b - a, H2], F32R, tag=f"w2c{ci}", name=f"w2c{ci}")
                nc.sync.dma_start(t[:], w2_d[a:b, :])
                w2c.append(t)
            wot = per.tile([128, 4], F32R, tag="wot")
            nc.sync.dma_start(wot[:], wot_d[:])
            v0 = per.tile([128, 1], F32R, tag="v0")
            nc.sync.dma_start(v0[:], v0_d[:])

            # all indices / continuous staged in one DMA each, tile-major
            idxsb = per.tile([128, NT * F], I32, tag="idxsb")
            nc.sync.dma_start(
                idxsb[:],
                bass.AP(idx_d.tensor, 0, [(F, 128), (128 * F, NT), (1, F)]))
            contsb = per.tile([128, NT * CONT], F32, tag="contsb")
            nc.sync.dma_start(
                contsb[:],
                bass.AP(cont_d.tensor, 0,
                        [(CONT, 128), (128 * CONT, NT), (1, CONT)]))

            # feature-major X: chunk c at cols [c*BC, (c+1)*BC)
            xt = per.tile([128, 4 * BC], F32R, tag="xt")
            outsb = per.tile([1, BC], F32, tag="outsb")

            kc1 = [b - a for (a, b) in K1]     # 128,128,128,75
            kc2 = [b - a for (a, b) in K2]     # 128,128,128,17
            mw1 = [b - a for (a, b) in M1]
            mw2 = [b - a for (a, b) in M2]

            for rep in range(REPEAT):
             for p in range(NP):
                t0 = 2 * p
                G = gp.tile([128, 2 * GP], F32, tag="g")
                g = G[:]
                # per-field row gathers (HW indirect DMA: one index per
                # partition, 128 descriptors per call)
                for k in range(2):
                    for f in range(F):
                        nc.gpsimd.indirect_dma_start(
                            out=G[:, k * GP + HD + GW * f:
                                  k * GP + HD + GW * (f + 1)],
                            out_offset=None,
                            in_=gtab[:],
                            in_offset=bass.IndirectOffsetOnAxis(
                                ap=idxsb[:, (t0 + k) * F + f:
                                         (t0 + k) * F + f + 1], axis=0),
                        )
                # header: continuous, ones, pads (pads/ones persist per buffer)
                nc.vector.tensor_copy(
                    _view(g, 0, GP, 2, CONT),
                    _view(contsb[:], t0 * CONT, CONT, 2, CONT))
                nc.vector.memset(_view(g, CONT, GP, 2, 1), 1.0)
                nc.vector.memset(_view(g, CONT + 2, GP, 2, 2), 0.0)
                nc.vector.memset(_view(g, GF, GP, 2, GP - GF), 0.0)

                # ---- FM (pair-wide 3D APs) ----
                s1 = fmp.tile([128, 442], F32, tag="s1")
                nc.vector.tensor_tensor(
                    _view(s1[:], 0, 221, 2, 221),
                    _view(g, HD, GP, 2, 221), _view(g, HD + 221, GP, 2, 221),
                    op=ALU.add)
                s2 = fmp.tile([128, 204], F32, tag="s2")
                nc.vector.tensor_tensor(
                    _view(s2[:], 0, 102, 2, 102),
                    _view(s1[:], 0, 221, 2, 102), _view(s1[:], 102, 221, 2, 102),
                    op=ALU.add)
                s3 = fmp.tile([128, 102], F32, tag="s3")
                nc.vector.tensor_tensor(
                    _view(s3[:], 0, 51, 2, 51),
                    _view(s2[:], 0, 102, 2, 51), _view(s2[:], 51, 102, 2, 51),
                    op=ALU.add)
                s6 = fmp.tile([128, 34], F32, tag="s6")
                s6v = _view(s6[:], 0, 17, 2, 17)
                nc.vector.tensor_tensor(
                    s6v, _view(s3[:], 0, 51, 2, 17), _view(s3[:], 17, 51, 2, 17),
                    op=ALU.add)
                nc.vector.tensor_tensor(
                    s6v, s6v, _view(s3[:], 34, 51, 2, 17), op=ALU.add)
                nc.vector.tensor_tensor(
                    s6v, s6v, _view(s1[:], 204, 221, 2, 17), op=ALU.add)
                # sum of squares over emb cols only (strided view skips first)
                sqscr = fmp.tile([128, F * E], F32, tag="sqscr")
                sqall = fmp.tile([128, 2], F32, tag="sqall")
                se2scr = fmp.tile([128, E], F32, tag="se2scr")
                se2r = fmp.tile([128, 2], F32, tag="se2r")
                for k in range(2):
                    pstep = g.ap[0][0]
                    embview = bass.AP(g.tensor, g.offset + k * GP + HD,
                                      [(pstep, 128), (GW, F), (1, E)])
                    nc.scalar.activation(sqscr[:], embview, AF.Square,
                                         accum_out=sqall[:, k:k + 1])
                    nc.scalar.activation(se2scr[:], s6[:, 17 * k:17 * k + E],
                                         AF.Square, accum_out=se2r[:, k:k + 1])
                t2 = fmp.tile([128, 2], F32, tag="t2")
                nc.vector.tensor_tensor(t2[:], se2r[:], sqall[:], op=ALU.subtract)
                # fm col = 0.5*t2 + sum_first
                fmcol = _view(g, CONT + 1, GP, 2, 1)
                nc.vector.tensor_scalar(
                    fmcol, _view(t2[:], 0, 1, 2, 1), 0.5, None, op0=ALU.mult)
                nc.vector.tensor_tensor(
                    fmcol, fmcol, _view(s6[:], 16, 17, 2, 1), op=ALU.add)

                # ---- transposes into feature-major xt ----
                for k in range(2):
                    t = t0 + k
                    tp = pst.tile([128, 512], F32, tag="tp")
                    for c in (1, 2, 3, 0):   # chunk 0 last (depends on fm col)
                        nc.tensor.transpose(
                            tp[:, 128 * c:128 * (c + 1)],
                            G[:, k * GP + 128 * c:k * GP + 128 * (c + 1)],
                            ident[:])
                    nc.vector.tensor_copy(
                        _view(xt[:], t * 128, BC, 4, 128), tp[:])

                # ---- MLP for b-tile bt after its two pairs ----
                if p % 2 == 1:
                    bt = (p - 1) // 2
                    bsl = slice(bt * 512, (bt + 1) * 512)
                    # layer 1
                    h1t = h1p.tile([128, 4 * 512], F32R, tag="h1t")
                    for mi, (ma, mb) in enumerate(M1):
                        ps = psm.tile([128, 512], F32, tag="ps")
                        for ci in range(4):
                            nc.tensor.matmul(
                                ps[0:mw1[mi], :],
                                lhsT=w1c[ci][:, ma:mb],
                                rhs=xt[0:kc1[ci],
                                       ci * BC + bt * 512:ci * BC + (bt + 1) * 512
                                       ],
                                start=(ci == 0), stop=(ci == 3))
                        dst = h1t[0:mw1[mi], 512 * mi:512 * mi + 512]
                        if mi < 2:
                            nc.vector.tensor_scalar_max(dst, ps[0:mw1[mi], :], 0.0)
                        else:
                            nc.scalar.activation(dst, ps[0:mw1[mi], :], AF.Relu)
                    # layer 2
                    h2t = h2p.tile([128, 4 * 512], F32R, tag="h2t")
                    for mi, (ma, mb) in enumerate(M2):
                        ps = psm.tile([128, 512], F32, tag="ps")
                        for ci in range(4):
                            nc.tensor.matmul(
                                ps[0:mw2[mi], :],
                                lhsT=w2c[ci][:, ma:mb],
                                rhs=h1t[0:kc2[ci],
                                        512 * ci:512 * ci + 512],
                                start=(ci == 0), stop=(ci == 3))
                        dst = h2t[0:mw2[mi], 512 * mi:512 * mi + 512]
                        if mi < 2:
                            nc.vector.tensor_scalar_max(dst, ps[0:mw2[mi], :], 0.0)
                        else:
                            nc.scalar.activation(dst, ps[0:mw2[mi], :], AF.Relu)
                    # final: out[1, 512] = v0.X0 + sum_c wot_c.h2_c
                    pf = psf.tile([1, 512], F32, tag="pf")
                    nc.tensor.matmul(
                        pf[:], lhsT=v0[:],
                        rhs=xt[:, bt * 512:(bt + 1) * 512],
                        start=True, stop=False)
                    for c in range(3):
                        nc.tensor.matmul(
                            pf[:], lhsT=wot[:, c:c + 1],
                            rhs=h2t[:, 512 * c:512 * c + 512],
                            start=False, stop=False)
                    nc.tensor.matmul(
                        pf[:], lhsT=wot[0:16, 3:4],
                        rhs=h2t[0:16, 1536:2048],
                        start=False, stop=True)
                    nc.vector.tensor_copy(outsb[:, bsl], pf[:])
            nc.sync.dma_start(out_d[:], outsb[:])
    nc.compile()
    return nc


class _Runner:
    def __init__(self, nc, n_cores, shared):
        install_neuronx_cc_hook()
        self.nc = nc
        self.n_cores = n_cores
        self.shared = set(shared)
        pname = nc.partition_id_tensor.name if nc.partition_id_tensor else None
        in_names, out_names, out_avals = [], [], []
        self.out_shapes = {}
        for alloc in nc.m.functions[0].allocations:
            if not isinstance(alloc, mybir.MemoryLocationSet):
                continue
            name = alloc.memorylocations[0].name
            if alloc.kind == "ExternalInput":
                if name != pname:
                    in_names.append(name)
            elif alloc.kind == "ExternalOutput":
                shape = tuple(alloc.tensor_shape)
                dtype = mybir.dt.np(alloc.dtype)
                out_names.append(name)
                out_avals.append(jax.core.ShapedArray(shape, dtype))
                self.out_shapes[name] = (shape, dtype)
        self.in_names, self.out_names = in_names, out_names
        all_in = in_names + out_names + ([pname] if pname else [])

        def _body(*args):
            ops = list(args)
            if pname:
                ops.append(partition_id_tensor())
            return tuple(_bass_exec_p.bind(
                *ops, out_avals=tuple(out_avals), in_names=tuple(all_in),
                out_names=tuple(out_names), lowering_input_output_aliases=(),
                sim_require_finite=True, sim_require_nnan=True, nc=nc))

        devs = jax.devices()[:n_cores]
        self.mesh = Mesh(np.asarray(devs), ("core",))
        in_specs = tuple(
            PartitionSpec(None) if nm in self.shared else PartitionSpec("core")
            for nm in in_names) + (PartitionSpec("core"),) * len(out_names)
        self.fn = jax.jit(
            shard_map(_body, mesh=self.mesh, in_specs=in_specs,
                      out_specs=(PartitionSpec("core"),) * len(out_names),
                      check_rep=False),
            keep_unused=True)

    def run(self, in_maps):
        args = []
        for nm in self.in_names:
            if nm in self.shared:
                a = np.ascontiguousarray(in_maps[0][nm])
                sh = NamedSharding(self.mesh, PartitionSpec(None))
            else:
                a = np.concatenate([np.asarray(m[nm]) for m in in_maps], axis=0)
                sh = NamedSharding(self.mesh, PartitionSpec("core"))
            args.append(jax.device_put(a, sh))
        for nm in self.out_names:
            shape, dtype = self.out_shapes[nm]
            z = np.zeros((self.n_cores * shape[0], *shape[1:]), dtype)
            args.append(jax.device_put(
                z, NamedSharding(self.mesh, PartitionSpec("core"))))
        outs = self.fn(*args)
        jax.block_until_ready(outs)
        self._last_args = args
        res = [dict() for _ in range(self.n_cores)]
        for i, nm in enumerate(self.out_names):
            shape, _ = self.out_shapes[nm]
            full = np.asarray(outs[i]).reshape(self.n_cores, *shape)
            for c in range(self.n_cores):
                res[c][nm] = full[c]
        return res


_CACHE = {}


def _prep_host(inputs):
    cat = np.asarray(inputs["categorical"]).astype(np.int64)
    cont = np.asarray(inputs["continuous"], dtype=np.float32)
    te = np.asarray(inputs["tables_emb"], dtype=np.float32)     # [F, V, 16]
    tf = np.asarray(inputs["tables_first"], dtype=np.float32)   # [F, V, 1]
    W1 = np.asarray(inputs["W1"], dtype=np.float32)
    b1 = np.asarray(inputs["b1"], dtype=np.float32)
    W2 = np.asarray(inputs["W2"], dtype=np.float32)
    b2 = np.asarray(inputs["b2"], dtype=np.float32)
    Wo = np.asarray(inputs["W_out"], dtype=np.float32)
    bo = np.asarray(inputs["b_out"], dtype=np.float32)
    wc = np.asarray(inputs["w_cont"], dtype=np.float32)
    bc = np.asarray(inputs["b_cont"], dtype=np.float32)

    gtab = np.concatenate(
        [te.reshape(F * V, E), tf.reshape(F * V, 1)], axis=1).astype(np.float32)
    flat = (cat + (np.arange(F, dtype=np.int64) * V)[None, :]).astype(np.int32)

    # W1 permuted to G-column order
    w1p = np.zeros((GF, H1 + 1), np.float32)
    w1p[0:CONT, 0:H1] = W1[0:CONT]
    w1p[13, 0:H1] = b1
    w1p[13, H1] = 1.0
    for f in range(F):
        w1p[HD + GW * f: HD + GW * f + E, 0:H1] = W1[CONT + E * f: CONT + E * (f + 1)]
    w2p = np.zeros((401, H2), np.float32)
    w2p[0:400] = W2
    w2p[400] = b2
    wo_pad = np.zeros(512, np.float32)
    wo_pad[:400] = Wo[1:, 0]
    wot = np.ascontiguousarray(wo_pad.reshape(4, 128).T)
    w00 = float(Wo[0, 0])
    v0 = np.zeros((128, 1), np.float32)
    v0[0:CONT, 0] = w00 * wc[:, 0]
    v0[13, 0] = float(bo[0]) + w00 * float(bc[0])
    v0[14, 0] = w00

    def _r(a):  # round to FP22 (fp32r): truncate low 10 mantissa bits
        return (a.view(np.uint32) & np.uint32(0xFFFFFC00)).view(np.float32)

    return gtab, flat, cont, _r(w1p), _r(w2p), _r(wot), _r(v0)


def kernel(**inputs) -> np.ndarray:
    gtab, flat, cont, w1p, w2p, wot, v0 = _prep_host(inputs)
    if "nc" not in _CACHE:
        _CACHE["nc"] = _build()
        _CACHE["runner"] = _Runner(
            _CACHE["nc"], NCORES,
            shared={"gtab", "w1_d", "w2_d", "wot_d", "v0_d"})
    r = _CACHE["runner"]
    in_maps = []
    for c in range(NCORES):
        sl = slice(c * BC, (c + 1) * BC)
        in_maps.append({
            "gtab": gtab, "w1_d": w1p, "w2_d": w2p, "wot_d": wot, "v0_d": v0,
            "idx_d": np.ascontiguousarray(flat[sl]),
            "cont_d": np.ascontiguousarray(cont[sl]),
        })
    res = r.run(in_maps)
    out = np.concatenate([res[c]["out_d"].reshape(BC) for c in range(NCORES)])
    return out.reshape(B, 1).astype(np.float32)

